# revision 10
# baseline (speedup 1.0000x reference)
"""Jansen-Rit neural-mass forward (Euler, per-step history) on 8 TRN2 cores.

Approach: each Euler step is S' = L S + G u with u = sigmoid(C S + b) (3
sigmoids/region); the only nonlinearity is the sigmoid. We process blocks of
K_BLK=40 steps at once via Picard iteration: guess the block's 120 sigmoid
values per region by quadratic extrapolation from the previous block, then
each sweep is ONE matmul against a precomputed block-propagation matrix
(TensorE) + ONE 120-partition sigmoid (ScalarE). Two sweeps reproduce the
exact sequential fp32 trajectory to ~1e-4 max-rel (validated vs fp64).
Regions (200) are sharded 25/core across 8 cores; history is written in
[comp*step_in_subblock, subblock, region] layout (contiguous DMA) and
permuted to [T, R, 6] on host.
"""

import os
import numpy as np

K_BLK = 40          # steps per Picard block
N_SWEEPS = 2        # sigmoid evaluations per block (incl. the extrapolated one)
N_CORES = 8
R_FULL = 200
R_CORE = R_FULL // N_CORES  # 25
NV = 3 * K_BLK + 7          # rhs vector length: [u(120); S(6); 1] = 127

_CACHE = {}
LAST_RESULTS = None  # BassKernelResults of the most recent device run


def _sig(x, vmax, v0, r):
    return vmax / (1.0 + np.exp(r * (v0 - x)))


def _seq_euler(state, n, dt, A, a, B, b, c1, c2, c3, c4, vmax, v0, r, std_in):
    """Sequential fp32 Euler, same op order as the reference; returns the
    states at steps 0..n-1 (i.e. including the initial state, excluding the
    state after step n)."""
    f = np.float32
    M, E, I, Mv, Ev, Iv = (state[:, i].astype(f).copy() for i in range(6))
    out = np.empty((n, state.shape[0], 6), f)
    for t in range(n):
        out[t] = np.stack([M, E, I, Mv, Ev, Iv], axis=1)
        sEI = _sig(E - I, f(vmax), f(v0), f(r)).astype(f)
        sM1 = _sig(f(c1) * M, f(vmax), f(v0), f(r)).astype(f)
        sM3 = _sig(f(c3) * M, f(vmax), f(v0), f(r)).astype(f)
        dMv = f(A * a) * sEI - f(2 * a) * Mv - M * f(a * a)
        dEv = f(A * a) * (f(std_in) + f(c2) * sM1) - f(2 * a) * Ev - E * f(a * a)
        dIv = f(B * b) * (f(c4) * sM3) - f(2 * b) * Iv - I * f(b * b)
        M = M + f(dt) * Mv
        E = E + f(dt) * Ev
        I = I + f(dt) * Iv
        Mv = Mv + f(dt) * dMv
        Ev = Ev + f(dt) * dEv
        Iv = Iv + f(dt) * dIv
    return out, np.stack([M, E, I, Mv, Ev, Iv], axis=1)


def _build_mats(dt, A, a, B, b, c1, c2, c3, c4, vmax, v0, r, std_in):
    """Block-propagation matrices (float64). Returns Z (126 x 127),
    P (120 x 127), Q (240 x 127)."""
    k = K_BLK
    L = np.zeros((6, 6))
    L[0, 0] = 1; L[0, 3] = dt
    L[1, 1] = 1; L[1, 4] = dt
    L[2, 2] = 1; L[2, 5] = dt
    L[3, 0] = -a * a * dt; L[3, 3] = 1 - 2 * a * dt
    L[4, 1] = -a * a * dt; L[4, 4] = 1 - 2 * a * dt
    L[5, 2] = -b * b * dt; L[5, 5] = 1 - 2 * b * dt
    Gu = np.zeros((6, 3))
    Gu[3, 0] = dt * A * a * vmax
    Gu[4, 1] = dt * A * a * c2 * vmax
    Gu[5, 2] = dt * B * b * c4 * vmax
    g1 = np.zeros(6)
    g1[4] = dt * A * a * std_in
    C = np.zeros((3, 6))
    C[0, 1] = r; C[0, 2] = -r
    C[1, 0] = c1 * r
    C[2, 0] = c3 * r

    Lp = [np.eye(6)]
    for _ in range(k + 1):
        Lp.append(Lp[-1] @ L)

    P = np.zeros((3 * k, NV))
    Q = np.zeros((6 * k, NV))
    for j in range(k):
        P[3 * j:3 * j + 3, 3 * k:3 * k + 6] = C @ Lp[j]
        for i in range(j):
            P[3 * j:3 * j + 3, 3 * i:3 * i + 3] = C @ Lp[j - 1 - i] @ Gu
            P[3 * j:3 * j + 3, 3 * k + 6] += C @ Lp[j - 1 - i] @ g1
        Q[6 * j:6 * j + 6, 3 * k:3 * k + 6] = Lp[j + 1]
        for i in range(j + 1):
            Q[6 * j:6 * j + 6, 3 * i:3 * i + 3] = Lp[j - i] @ Gu
            Q[6 * j:6 * j + 6, 3 * k + 6] += Lp[j - i] @ g1

    # quadratic least-squares extrapolation over the previous block's k points
    xs = np.arange(k)
    V = np.vander(xs, 3, increasing=True)
    W = np.vander(np.arange(k, 2 * k), 3, increasing=True) @ np.linalg.pinv(V)
    E_pad = np.zeros((3 * k, NV))
    for q in range(3):
        E_pad[q::3, q:3 * k:3] = W
    A_s = Q[6 * (k - 1):6 * k]  # S_{t0+k} from v  (6 x NV)
    Zx = P[:, :3 * k] @ E_pad[:, :3 * k] @ np.eye(3 * k, NV) \
        + P[:, 3 * k:3 * k + 6] @ A_s
    Zx[:, 3 * k + 6] += P[:, 3 * k + 6]
    ones_row = np.zeros((1, NV)); ones_row[0, 3 * k + 6] = 1.0
    Z = np.vstack([Zx, A_s, ones_row])  # 127 x 127: [X(120); S(6); 1]
    return Z, P, Q


def _get_program(nb, reps=1):
    """Build + compile the Bass program for nb blocks (cached). reps>1
    repeats the whole computation (same I/O) for differential timing."""
    key = (nb, reps)
    if key in _CACHE:
        return _CACHE[key]
    from contextlib import ExitStack

    import concourse.bass as bass  # noqa: F401
    import concourse.mybir as mybir
    import concourse.tile as tile
    from concourse import bacc

    f32 = mybir.dt.float32
    nc = bacc.Bacc("TRN2", target_bir_lowering=False, debug=False,
                   enable_asserts=False, num_devices=N_CORES)
    R = R_CORE
    v0_d = nc.dram_tensor("v0", [NV, R], f32, kind="ExternalInput")
    zw_d = nc.dram_tensor("zw", [NV, 127], f32, kind="ExternalInput")
    pw_d = nc.dram_tensor("pw", [NV, 120], f32, kind="ExternalInput")
    q1_d = nc.dram_tensor("q1", [NV, 120], f32, kind="ExternalInput")
    q2_d = nc.dram_tensor("q2", [NV, 120], f32, kind="ExternalInput")
    bias_d = nc.dram_tensor("bias", [120, 1], f32, kind="ExternalInput")
    nsb = 2 * nb  # sub-blocks of 20 steps
    hist_d = nc.dram_tensor("hist", [120, nsb * R], f32, kind="ExternalOutput")

    CHUNK = 25  # blocks per history DMA (50 sub-blocks)
    assert nb % CHUNK == 0

    with tile.TileContext(nc) as tc, ExitStack() as ctx:
        sb = ctx.enter_context(tc.tile_pool(name="persist", bufs=1))
        stage_p = ctx.enter_context(tc.tile_pool(name="stage", bufs=2))
        ps_z = ctx.enter_context(tc.tile_pool(name="psz", bufs=1, space="PSUM"))
        ps_p = ctx.enter_context(tc.tile_pool(name="psp", bufs=1, space="PSUM"))
        ps_h = ctx.enter_context(tc.tile_pool(name="psh", bufs=4, space="PSUM"))

        vts = [sb.tile([NV, R], f32, tag="va", name="va"),
               sb.tile([NV, R], f32, tag="vb", name="vb")]
        zw = sb.tile([NV, 127], f32, tag="zw")
        pw = sb.tile([NV, 120], f32, tag="pw")
        q1 = sb.tile([NV, 120], f32, tag="q1")
        q2 = sb.tile([NV, 120], f32, tag="q2")
        bias = sb.tile([120, 1], f32, tag="bias")
        nc.sync.dma_start(zw[:], zw_d.ap())
        nc.sync.dma_start(pw[:], pw_d.ap())
        nc.sync.dma_start(q1[:], q1_d.ap())
        nc.sync.dma_start(q2[:], q2_d.ap())
        nc.sync.dma_start(bias[:], bias_d.ap())

        sigmoid = mybir.ActivationFunctionType.Sigmoid
        for rep in range(reps):
            nc.sync.dma_start(vts[0][:], v0_d.ap())
            _emit_blocks(nc, tc, nb, CHUNK, R, f32, sigmoid,
                         vts, zw, pw, q1, q2, bias, stage_p,
                         ps_z, ps_p, ps_h, hist_d)

    nc.compile()
    _CACHE[key] = nc
    return nc


def _emit_blocks(nc, tc, nb, CHUNK, R, f32, sigmoid, vts, zw, pw, q1, q2,
                 bias, stage_p, ps_z, ps_p, ps_h, hist_d):
        for chunk in range(nb // CHUNK):
            stage = stage_p.tile([120, CHUNK * 2 * R], f32, tag="stage")
            for j in range(CHUNK):
                b = chunk * CHUNK + j
                vcur = vts[b % 2]
                vnxt = vts[(b + 1) % 2]
                if b < nb - 1:
                    # sweep 1: advance state + extrapolated-sigmoid correction
                    # pz rows: [X(120); S_next(6); 1]
                    pz = ps_z.tile([NV, R], f32, tag="pz")
                    nc.tensor.matmul(pz[:], zw[:], vcur[:], start=True, stop=True)
                    # S+ones into rows 96:127 of vnxt (base-96 AP; rows 96:120
                    # are overwritten by the activation right after)
                    nc.vector.tensor_copy(vnxt[96:NV, :], pz[96:NV, :])
                    nc.scalar.activation(vnxt[0:120, :], pz[0:120, :],
                                         sigmoid, bias=bias[:], scale=1.0)
                # history of block b (from the converged v_b)
                ph1 = ps_h.tile([120, R], f32, tag="ph")
                nc.tensor.matmul(ph1[:], q1[:], vcur[:], start=True, stop=True)
                nc.vector.tensor_copy(stage[:, (2 * j) * R:(2 * j + 1) * R], ph1[:])
                ph2 = ps_h.tile([120, R], f32, tag="ph")
                nc.tensor.matmul(ph2[:], q2[:], vcur[:], start=True, stop=True)
                nc.vector.tensor_copy(stage[:, (2 * j + 1) * R:(2 * j + 2) * R], ph2[:])
                if b < nb - 1:
                    # sweep 2
                    pp = ps_p.tile([120, R], f32, tag="pp")
                    nc.tensor.matmul(pp[:], pw[:], vnxt[:], start=True, stop=True)
                    nc.scalar.activation(vnxt[0:120, :], pp[:],
                                         sigmoid, bias=bias[:], scale=1.0)
            nc.sync.dma_start(
                hist_d.ap()[:, chunk * CHUNK * 2 * R:(chunk + 1) * CHUNK * 2 * R],
                stage[:])


def kernel(init_state, step_size, A, a, B, b, c1, c2, c3, c4,
           vmax, v0, r, std_in, num_steps):
    global LAST_RESULTS
    init_state = np.asarray(init_state, np.float32)
    pars = [float(np.asarray(x)) for x in
            (step_size, A, a, B, b, c1, c2, c3, c4, vmax, v0, r, std_in)]
    (dt, A_, a_, B_, b_, c1_, c2_, c3_, c4_, vmax_, v0_, r_, std_in_) = pars
    T = int(np.asarray(num_steps))
    Rf = init_state.shape[0]

    k = K_BLK
    if Rf != R_FULL or T <= 2 * k:
        # tiny/odd shapes: host fallback (exact sequential)
        hist, _ = _seq_euler(init_state, T + 1, *pars)
        state_hist = hist[1:T + 1]
        return state_hist[-1].copy(), state_hist

    # pad T up to a multiple of CHUNK*K_BLK (=1000)
    blk_quant = 25 * k
    T_pad = -(-T // blk_quant) * blk_quant
    nb = T_pad // k

    Z, P, Q = _build_mats(dt, A_, a_, B_, b_, c1_, c2_, c3_, c4_,
                          vmax_, v0_, r_, std_in_)
    Zt = np.ascontiguousarray(Z.T, np.float32)
    Pt = np.ascontiguousarray(P.T, np.float32)
    Q1t = np.ascontiguousarray(Q[:120].T, np.float32)
    Q2t = np.ascontiguousarray(Q[120:].T, np.float32)
    bias_np = np.full((120, 1), -r_ * v0_, np.float32)

    # bootstrap block 0 on host (exact fp32 sequential, k steps)
    states0, _ = _seq_euler(init_state, k, *pars)  # states at steps 0..k-1
    u0 = np.empty((3 * k, Rf), np.float32)
    for j in range(k):
        M = states0[j, :, 0]; E = states0[j, :, 1]; I = states0[j, :, 2]
        u0[3 * j + 0] = 1.0 / (1.0 + np.exp(-(r_ * (E - I) - r_ * v0_)))
        u0[3 * j + 1] = 1.0 / (1.0 + np.exp(-(c1_ * r_ * M - r_ * v0_)))
        u0[3 * j + 2] = 1.0 / (1.0 + np.exp(-(c3_ * r_ * M - r_ * v0_)))
    v0_full = np.concatenate(
        [u0, init_state.T.astype(np.float32), np.ones((1, Rf), np.float32)], 0)

    reps = int(os.environ.get("KERNEL_REPS", "1"))
    nc = _get_program(nb, reps)
    in_maps = []
    for c in range(N_CORES):
        sl = slice(c * R_CORE, (c + 1) * R_CORE)
        in_maps.append({
            "v0": np.ascontiguousarray(v0_full[:, sl]),
            "zw": Zt, "pw": Pt, "q1": Q1t, "q2": Q2t, "bias": bias_np,
        })

    from concourse import bass_utils
    res = bass_utils.run_bass_kernel_spmd(
        nc, in_maps, core_ids=list(range(N_CORES)),
        trace=bool(int(os.environ.get("KERNEL_TRACE", "0"))))
    LAST_RESULTS = res

    nsb = 2 * nb
    parts = []
    for c in range(N_CORES):
        arr = res.results[c]["hist"].reshape(20, 6, nsb, R_CORE)
        parts.append(arr.transpose(2, 0, 3, 1).reshape(nsb * 20, R_CORE, 6))
    state_hist = np.concatenate(parts, axis=1)[:T]
    return state_hist[-1].copy(), state_hist


# revision 15
# speedup vs baseline: 1.7091x; 1.7091x over previous
"""Jansen-Rit neural-mass forward (Euler, per-step history) on 8 TRN2 cores.

Approach: each Euler step is S' = L S + G u with u = sigmoid(C S + b) (3
sigmoids/region); the only nonlinearity is the sigmoid. We process blocks of
K_BLK=40 steps at once via Picard iteration: guess the block's 120 sigmoid
values per region by quadratic extrapolation from the previous block, then
each sweep is ONE matmul against a precomputed block-propagation matrix
(TensorE) + ONE 120-partition sigmoid (ScalarE). Two sweeps reproduce the
exact sequential fp32 trajectory to ~1e-4 max-rel (validated vs fp64).
Regions (200) are sharded 25/core across 8 cores; history is written in
[comp*step_in_subblock, subblock, region] layout (contiguous DMA) and
permuted to [T, R, 6] on host.
"""

import os
import numpy as np

K_BLK = 40          # steps per Picard block
N_SWEEPS = 2        # sigmoid evaluations per block (incl. the extrapolated one)
N_CORES = 8
R_FULL = 200
R_CORE = R_FULL // N_CORES  # 25
NV = 3 * K_BLK + 7          # rhs vector length: [u(120); S(6); 1] = 127

_CACHE = {}
LAST_RESULTS = None  # BassKernelResults of the most recent device run


def _sig(x, vmax, v0, r):
    return vmax / (1.0 + np.exp(r * (v0 - x)))


def _seq_euler(state, n, dt, A, a, B, b, c1, c2, c3, c4, vmax, v0, r, std_in):
    """Sequential fp32 Euler, same op order as the reference; returns the
    states at steps 0..n-1 (i.e. including the initial state, excluding the
    state after step n)."""
    f = np.float32
    M, E, I, Mv, Ev, Iv = (state[:, i].astype(f).copy() for i in range(6))
    out = np.empty((n, state.shape[0], 6), f)
    for t in range(n):
        out[t] = np.stack([M, E, I, Mv, Ev, Iv], axis=1)
        sEI = _sig(E - I, f(vmax), f(v0), f(r)).astype(f)
        sM1 = _sig(f(c1) * M, f(vmax), f(v0), f(r)).astype(f)
        sM3 = _sig(f(c3) * M, f(vmax), f(v0), f(r)).astype(f)
        dMv = f(A * a) * sEI - f(2 * a) * Mv - M * f(a * a)
        dEv = f(A * a) * (f(std_in) + f(c2) * sM1) - f(2 * a) * Ev - E * f(a * a)
        dIv = f(B * b) * (f(c4) * sM3) - f(2 * b) * Iv - I * f(b * b)
        M = M + f(dt) * Mv
        E = E + f(dt) * Ev
        I = I + f(dt) * Iv
        Mv = Mv + f(dt) * dMv
        Ev = Ev + f(dt) * dEv
        Iv = Iv + f(dt) * dIv
    return out, np.stack([M, E, I, Mv, Ev, Iv], axis=1)


def _build_mats(dt, A, a, B, b, c1, c2, c3, c4, vmax, v0, r, std_in):
    """Block-propagation matrices (float64). Returns Z (126 x 127),
    P (120 x 127), Q (240 x 127)."""
    k = K_BLK
    L = np.zeros((6, 6))
    L[0, 0] = 1; L[0, 3] = dt
    L[1, 1] = 1; L[1, 4] = dt
    L[2, 2] = 1; L[2, 5] = dt
    L[3, 0] = -a * a * dt; L[3, 3] = 1 - 2 * a * dt
    L[4, 1] = -a * a * dt; L[4, 4] = 1 - 2 * a * dt
    L[5, 2] = -b * b * dt; L[5, 5] = 1 - 2 * b * dt
    Gu = np.zeros((6, 3))
    Gu[3, 0] = dt * A * a * vmax
    Gu[4, 1] = dt * A * a * c2 * vmax
    Gu[5, 2] = dt * B * b * c4 * vmax
    g1 = np.zeros(6)
    g1[4] = dt * A * a * std_in
    C = np.zeros((3, 6))
    C[0, 1] = r; C[0, 2] = -r
    C[1, 0] = c1 * r
    C[2, 0] = c3 * r

    Lp = [np.eye(6)]
    for _ in range(k + 1):
        Lp.append(Lp[-1] @ L)

    P = np.zeros((3 * k, NV))
    Q = np.zeros((6 * k, NV))
    for j in range(k):
        P[3 * j:3 * j + 3, 3 * k:3 * k + 6] = C @ Lp[j]
        for i in range(j):
            P[3 * j:3 * j + 3, 3 * i:3 * i + 3] = C @ Lp[j - 1 - i] @ Gu
            P[3 * j:3 * j + 3, 3 * k + 6] += C @ Lp[j - 1 - i] @ g1
        Q[6 * j:6 * j + 6, 3 * k:3 * k + 6] = Lp[j + 1]
        for i in range(j + 1):
            Q[6 * j:6 * j + 6, 3 * i:3 * i + 3] = Lp[j - i] @ Gu
            Q[6 * j:6 * j + 6, 3 * k + 6] += Lp[j - i] @ g1

    # quartic least-squares extrapolation over the previous block's k points
    # (validated: k=40, 1 Picard sweep, order 4 -> 1.9e-4 max-rel vs fp64)
    xs = np.arange(k)
    V = np.vander(xs, 5, increasing=True)
    W = np.vander(np.arange(k, 2 * k), 5, increasing=True) @ np.linalg.pinv(V)
    E_pad = np.zeros((3 * k, NV))
    for q in range(3):
        E_pad[q::3, q:3 * k:3] = W
    A_s = Q[6 * (k - 1):6 * k]  # S_{t0+k} from v  (6 x NV)
    Zx = P[:, :3 * k] @ E_pad[:, :3 * k] @ np.eye(3 * k, NV) \
        + P[:, 3 * k:3 * k + 6] @ A_s
    Zx[:, 3 * k + 6] += P[:, 3 * k + 6]
    ones_row = np.zeros((1, NV)); ones_row[0, 3 * k + 6] = 1.0
    Z = np.vstack([Zx, A_s, ones_row])  # 127 x 127: [X(120); S(6); 1]
    return Z, P, Q


def _get_program(nb, reps=1):
    """Build + compile the Bass program for nb blocks (cached). reps>1
    repeats the whole computation (same I/O) for differential timing."""
    key = (nb, reps)
    if key in _CACHE:
        return _CACHE[key]
    from contextlib import ExitStack

    import concourse.bass as bass  # noqa: F401
    import concourse.mybir as mybir
    import concourse.tile as tile
    from concourse import bacc

    f32 = mybir.dt.float32
    nc = bacc.Bacc("TRN2", target_bir_lowering=False, debug=False,
                   enable_asserts=False, num_devices=N_CORES)
    R = R_CORE
    v0_d = nc.dram_tensor("v0", [NV, R], f32, kind="ExternalInput")
    zw_d = nc.dram_tensor("zw", [NV, 127], f32, kind="ExternalInput")
    q1_d = nc.dram_tensor("q1", [NV, 120], f32, kind="ExternalInput")
    q2_d = nc.dram_tensor("q2", [NV, 120], f32, kind="ExternalInput")
    bias_d = nc.dram_tensor("bias", [120, 1], f32, kind="ExternalInput")
    GRP = 4            # blocks per history matmul batch
    GCHUNK = 25        # groups per history DMA
    assert nb % (GRP * GCHUNK) == 0
    ngrp = nb // GRP
    # hist col layout: per group g: [Q1 out for blocks 4g..4g+3 | Q2 same]
    hist_d = nc.dram_tensor("hist", [120, ngrp * 2 * GRP * R], f32,
                            kind="ExternalOutput")

    with tile.TileContext(nc) as tc, ExitStack() as ctx:
        sb = ctx.enter_context(tc.tile_pool(name="persist", bufs=1))
        stage_p = ctx.enter_context(tc.tile_pool(name="stage", bufs=2))
        ps_z = ctx.enter_context(tc.tile_pool(name="psz", bufs=2, space="PSUM"))
        ps_h = ctx.enter_context(tc.tile_pool(name="psh", bufs=2, space="PSUM"))

        va = sb.tile([NV, 8 * R], f32, tag="va", name="va")  # 8-slot v ring
        zw = sb.tile([NV, 127], f32, tag="zw")
        q1 = sb.tile([NV, 120], f32, tag="q1")
        q2 = sb.tile([NV, 120], f32, tag="q2")
        bias = sb.tile([120, 1], f32, tag="bias")
        nc.sync.dma_start(zw[:], zw_d.ap())
        nc.sync.dma_start(q1[:], q1_d.ap())
        nc.sync.dma_start(q2[:], q2_d.ap())
        nc.sync.dma_start(bias[:], bias_d.ap())

        sigmoid = mybir.ActivationFunctionType.Sigmoid
        GW = 2 * GRP * R  # hist cols per group (200)
        for rep in range(reps):
            nc.sync.dma_start(va[:, 0:R], v0_d.ap())
            stage = None
            for b in range(nb):
                vcur = va[:, (b % 8) * R:(b % 8) * R + R]
                if b < nb - 1:
                    nxt = ((b + 1) % 8) * R
                    # one Picard sweep: X = Z.[u;S;1] with order-4
                    # extrapolation folded into Z; rows [X(120); S'(6); 1]
                    pz = ps_z.tile([NV, R], f32, tag="pz")
                    nc.tensor.matmul(pz[:], zw[:], vcur, start=True, stop=True)
                    # S'+1 -> rows 96:127 (base-96 AP rule); rows 96:120 are
                    # re-written by the activation just after (WAW ordered)
                    nc.vector.tensor_copy(va[96:NV, nxt:nxt + R], pz[96:NV, :])
                    nc.scalar.activation(va[0:120, nxt:nxt + R], pz[0:120, :],
                                         sigmoid, bias=bias[:], scale=1.0)
                if b % GRP == GRP - 1:
                    # batched history for blocks 4g..4g+3 (v slots contiguous)
                    g = b // GRP
                    if g % GCHUNK == 0:
                        stage = stage_p.tile([120, GCHUNK * GW], f32,
                                             tag="stage", name="stage")
                    s0 = ((b - (GRP - 1)) % 8) * R   # 0 or 100
                    c0 = (g % GCHUNK) * GW
                    ph1 = ps_h.tile([120, GRP * R], f32, tag="ph", name="ph1")
                    nc.tensor.matmul(ph1[:], q1[:], va[:, s0:s0 + GRP * R],
                                     start=True, stop=True)
                    nc.vector.tensor_copy(stage[:, c0:c0 + GRP * R], ph1[:])
                    ph2 = ps_h.tile([120, GRP * R], f32, tag="ph", name="ph2")
                    nc.tensor.matmul(ph2[:], q2[:], va[:, s0:s0 + GRP * R],
                                     start=True, stop=True)
                    nc.vector.tensor_copy(
                        stage[:, c0 + GRP * R:c0 + 2 * GRP * R], ph2[:])
                    if g % GCHUNK == GCHUNK - 1:
                        ch = g // GCHUNK
                        nc.sync.dma_start(
                            hist_d.ap()[:, ch * GCHUNK * GW:(ch + 1) * GCHUNK * GW],
                            stage[:])

    nc.compile()
    _CACHE[key] = nc
    return nc


def kernel(init_state, step_size, A, a, B, b, c1, c2, c3, c4,
           vmax, v0, r, std_in, num_steps):
    global LAST_RESULTS
    init_state = np.asarray(init_state, np.float32)
    pars = [float(np.asarray(x)) for x in
            (step_size, A, a, B, b, c1, c2, c3, c4, vmax, v0, r, std_in)]
    (dt, A_, a_, B_, b_, c1_, c2_, c3_, c4_, vmax_, v0_, r_, std_in_) = pars
    T = int(np.asarray(num_steps))
    Rf = init_state.shape[0]

    k = K_BLK
    if Rf != R_FULL or T <= 2 * k:
        # tiny/odd shapes: host fallback (exact sequential)
        hist, _ = _seq_euler(init_state, T + 1, *pars)
        state_hist = hist[1:T + 1]
        return state_hist[-1].copy(), state_hist

    # pad T up to a multiple of GRP*GCHUNK*K_BLK (=4000)
    blk_quant = 100 * k
    T_pad = -(-T // blk_quant) * blk_quant
    nb = T_pad // k

    Z, P, Q = _build_mats(dt, A_, a_, B_, b_, c1_, c2_, c3_, c4_,
                          vmax_, v0_, r_, std_in_)
    Zt = np.ascontiguousarray(Z.T, np.float32)
    Q1t = np.ascontiguousarray(Q[:120].T, np.float32)
    Q2t = np.ascontiguousarray(Q[120:].T, np.float32)
    bias_np = np.full((120, 1), -r_ * v0_, np.float32)

    # bootstrap block 0 on host (exact fp32 sequential, k steps)
    states0, _ = _seq_euler(init_state, k, *pars)  # states at steps 0..k-1
    u0 = np.empty((3 * k, Rf), np.float32)
    for j in range(k):
        M = states0[j, :, 0]; E = states0[j, :, 1]; I = states0[j, :, 2]
        u0[3 * j + 0] = 1.0 / (1.0 + np.exp(-(r_ * (E - I) - r_ * v0_)))
        u0[3 * j + 1] = 1.0 / (1.0 + np.exp(-(c1_ * r_ * M - r_ * v0_)))
        u0[3 * j + 2] = 1.0 / (1.0 + np.exp(-(c3_ * r_ * M - r_ * v0_)))
    v0_full = np.concatenate(
        [u0, init_state.T.astype(np.float32), np.ones((1, Rf), np.float32)], 0)

    reps = int(os.environ.get("KERNEL_REPS", "1"))
    nc = _get_program(nb, reps)
    in_maps = []
    for c in range(N_CORES):
        sl = slice(c * R_CORE, (c + 1) * R_CORE)
        in_maps.append({
            "v0": np.ascontiguousarray(v0_full[:, sl]),
            "zw": Zt, "q1": Q1t, "q2": Q2t, "bias": bias_np,
        })

    from concourse import bass_utils
    res = bass_utils.run_bass_kernel_spmd(
        nc, in_maps, core_ids=list(range(N_CORES)),
        trace=bool(int(os.environ.get("KERNEL_TRACE", "0"))))
    LAST_RESULTS = res

    ngrp = nb // 4
    parts = []
    for c in range(N_CORES):
        # hist cols: [g, half, i, r]; rows: [j, comp]
        arr = res.results[c]["hist"].reshape(20, 6, ngrp, 2, 4, R_CORE)
        # -> [g, i, half, j, r, comp]: t = (4g+i)*40 + half*20 + j
        parts.append(arr.transpose(2, 4, 3, 0, 5, 1).reshape(
            ngrp * 4 * 40, R_CORE, 6))
    state_hist = np.concatenate(parts, axis=1)[:T]
    return state_hist[-1].copy(), state_hist


# revision 20
# speedup vs baseline: 1.8156x; 1.0623x over previous
"""Jansen-Rit neural-mass forward (Euler, per-step history) on 8 TRN2 cores.

Approach: each Euler step is S' = L S + G u with u = sigmoid(C S + b) (3
sigmoids/region); the only nonlinearity is the sigmoid. We process blocks of
K_BLK=40 steps at once via Picard iteration: guess the block's 120 sigmoid
values per region by quadratic extrapolation from the previous block, then
each sweep is ONE matmul against a precomputed block-propagation matrix
(TensorE) + ONE 120-partition sigmoid (ScalarE). Two sweeps reproduce the
exact sequential fp32 trajectory to ~1e-4 max-rel (validated vs fp64).
Regions (200) are sharded 25/core across 8 cores; history is written in
[comp*step_in_subblock, subblock, region] layout (contiguous DMA) and
permuted to [T, R, 6] on host.
"""

import os
import numpy as np

K_BLK = 40          # steps per Picard block
N_SWEEPS = 2        # sigmoid evaluations per block (incl. the extrapolated one)
N_CORES = 8
R_FULL = 200
R_CORE = R_FULL // N_CORES  # 25
NV = 3 * K_BLK + 7          # rhs vector length: [u(120); S(6); 1] = 127

_CACHE = {}
LAST_RESULTS = None  # BassKernelResults of the most recent device run


def _sig(x, vmax, v0, r):
    return vmax / (1.0 + np.exp(r * (v0 - x)))


def _seq_euler(state, n, dt, A, a, B, b, c1, c2, c3, c4, vmax, v0, r, std_in):
    """Sequential fp32 Euler, same op order as the reference; returns the
    states at steps 0..n-1 (i.e. including the initial state, excluding the
    state after step n)."""
    f = np.float32
    M, E, I, Mv, Ev, Iv = (state[:, i].astype(f).copy() for i in range(6))
    out = np.empty((n, state.shape[0], 6), f)
    for t in range(n):
        out[t] = np.stack([M, E, I, Mv, Ev, Iv], axis=1)
        sEI = _sig(E - I, f(vmax), f(v0), f(r)).astype(f)
        sM1 = _sig(f(c1) * M, f(vmax), f(v0), f(r)).astype(f)
        sM3 = _sig(f(c3) * M, f(vmax), f(v0), f(r)).astype(f)
        dMv = f(A * a) * sEI - f(2 * a) * Mv - M * f(a * a)
        dEv = f(A * a) * (f(std_in) + f(c2) * sM1) - f(2 * a) * Ev - E * f(a * a)
        dIv = f(B * b) * (f(c4) * sM3) - f(2 * b) * Iv - I * f(b * b)
        M = M + f(dt) * Mv
        E = E + f(dt) * Ev
        I = I + f(dt) * Iv
        Mv = Mv + f(dt) * dMv
        Ev = Ev + f(dt) * dEv
        Iv = Iv + f(dt) * dIv
    return out, np.stack([M, E, I, Mv, Ev, Iv], axis=1)


def _build_mats(dt, A, a, B, b, c1, c2, c3, c4, vmax, v0, r, std_in):
    """Block-propagation matrices (float64). Returns Z (126 x 127),
    P (120 x 127), Q (240 x 127)."""
    k = K_BLK
    L = np.zeros((6, 6))
    L[0, 0] = 1; L[0, 3] = dt
    L[1, 1] = 1; L[1, 4] = dt
    L[2, 2] = 1; L[2, 5] = dt
    L[3, 0] = -a * a * dt; L[3, 3] = 1 - 2 * a * dt
    L[4, 1] = -a * a * dt; L[4, 4] = 1 - 2 * a * dt
    L[5, 2] = -b * b * dt; L[5, 5] = 1 - 2 * b * dt
    Gu = np.zeros((6, 3))
    Gu[3, 0] = dt * A * a * vmax
    Gu[4, 1] = dt * A * a * c2 * vmax
    Gu[5, 2] = dt * B * b * c4 * vmax
    g1 = np.zeros(6)
    g1[4] = dt * A * a * std_in
    C = np.zeros((3, 6))
    C[0, 1] = r; C[0, 2] = -r
    C[1, 0] = c1 * r
    C[2, 0] = c3 * r

    Lp = [np.eye(6)]
    for _ in range(k + 1):
        Lp.append(Lp[-1] @ L)

    P = np.zeros((3 * k, NV))
    Q = np.zeros((6 * k, NV))
    for j in range(k):
        P[3 * j:3 * j + 3, 3 * k:3 * k + 6] = C @ Lp[j]
        for i in range(j):
            P[3 * j:3 * j + 3, 3 * i:3 * i + 3] = C @ Lp[j - 1 - i] @ Gu
            P[3 * j:3 * j + 3, 3 * k + 6] += C @ Lp[j - 1 - i] @ g1
        Q[6 * j:6 * j + 6, 3 * k:3 * k + 6] = Lp[j + 1]
        for i in range(j + 1):
            Q[6 * j:6 * j + 6, 3 * i:3 * i + 3] = Lp[j - i] @ Gu
            Q[6 * j:6 * j + 6, 3 * k + 6] += Lp[j - i] @ g1

    # quartic least-squares extrapolation over the previous block's k points
    # (validated: k=40, 1 Picard sweep, order 4 -> 1.9e-4 max-rel vs fp64)
    xs = np.arange(k)
    V = np.vander(xs, 5, increasing=True)
    W = np.vander(np.arange(k, 2 * k), 5, increasing=True) @ np.linalg.pinv(V)
    E_pad = np.zeros((3 * k, NV))
    for q in range(3):
        E_pad[q::3, q:3 * k:3] = W
    A_s = Q[6 * (k - 1):6 * k]  # S_{t0+k} from v  (6 x NV)
    Zx = P[:, :3 * k] @ E_pad[:, :3 * k] @ np.eye(3 * k, NV) \
        + P[:, 3 * k:3 * k + 6] @ A_s
    Zx[:, 3 * k + 6] += P[:, 3 * k + 6]
    ones_row = np.zeros((1, NV)); ones_row[0, 3 * k + 6] = 1.0
    Z = np.vstack([Zx, A_s, ones_row])  # 127 x 127: [X(120); S(6); 1]
    return Z, P, Q


def _get_program(nb, reps=1):
    """Build + compile the Bass program for nb blocks (cached). reps>1
    repeats the whole computation (same I/O) for differential timing."""
    key = (nb, reps)
    if key in _CACHE:
        return _CACHE[key]
    from contextlib import ExitStack

    import concourse.bass as bass  # noqa: F401
    import concourse.mybir as mybir
    import concourse.tile as tile
    from concourse import bacc

    f32 = mybir.dt.float32
    nc = bacc.Bacc("TRN2", target_bir_lowering=False, debug=False,
                   enable_asserts=False, num_devices=N_CORES)
    R = R_CORE
    bf16 = mybir.dt.bfloat16
    v0_d = nc.dram_tensor("v0", [NV, R], f32, kind="ExternalInput")
    zw_d = nc.dram_tensor("zw", [NV, 127], f32, kind="ExternalInput")
    q1_d = nc.dram_tensor("q1", [NV, 120], bf16, kind="ExternalInput")
    q2_d = nc.dram_tensor("q2", [NV, 120], bf16, kind="ExternalInput")
    bias_d = nc.dram_tensor("bias", [120, 1], f32, kind="ExternalInput")
    GRP = 4            # blocks per history matmul batch
    GCHUNK = 25        # groups per history DMA
    assert nb % (GRP * GCHUNK) == 0
    ngrp = nb // GRP
    # hist col layout: per group g: [Q1 out for blocks 4g..4g+3 | Q2 same]
    hist_d = nc.dram_tensor("hist", [120, ngrp * 2 * GRP * R], f32,
                            kind="ExternalOutput")

    with tile.TileContext(nc) as tc, ExitStack() as ctx:
        sb = ctx.enter_context(tc.tile_pool(name="persist", bufs=1))
        stage_p = ctx.enter_context(tc.tile_pool(name="stage", bufs=2))
        ps_z = ctx.enter_context(tc.tile_pool(name="psz", bufs=2, space="PSUM"))
        ps_h = ctx.enter_context(tc.tile_pool(name="psh", bufs=2, space="PSUM"))

        va = sb.tile([NV, 8 * R], f32, tag="va", name="va")  # 8-slot v ring
        vbf = sb.tile([NV, 8 * R], bf16, tag="vbf", name="vbf")  # bf16 mirror
        zw = sb.tile([NV, 127], f32, tag="zw")
        q1 = sb.tile([NV, 120], bf16, tag="q1")
        q2 = sb.tile([NV, 120], bf16, tag="q2")
        bias = sb.tile([120, 1], f32, tag="bias")
        nc.sync.dma_start(zw[:], zw_d.ap())
        nc.sync.dma_start(q1[:], q1_d.ap())
        nc.sync.dma_start(q2[:], q2_d.ap())
        nc.sync.dma_start(bias[:], bias_d.ap())

        sigmoid = mybir.ActivationFunctionType.Sigmoid
        GW = 2 * GRP * R  # hist cols per group (200)
        for rep in range(reps):
            nc.sync.dma_start(va[:, 0:R], v0_d.ap())
            nc.vector.tensor_copy(vbf[:, 0:R], va[:, 0:R])
            stage = None
            for b in range(nb):
                vcur = va[:, (b % 8) * R:(b % 8) * R + R]
                if b < nb - 1:
                    nxt = ((b + 1) % 8) * R
                    # one Picard sweep: X = Z.[u;S;1] with order-4
                    # extrapolation folded into Z; rows [X(120); S'(6); 1]
                    pz = ps_z.tile([NV, R], f32, tag="pz")
                    nc.tensor.matmul(pz[:], zw[:], vcur, start=True, stop=True)
                    # S'+1 -> rows 96:127 (base-96 AP rule); rows 96:120 are
                    # re-written by the activation just after (WAW ordered)
                    nc.vector.tensor_copy(va[96:NV, nxt:nxt + R], pz[96:NV, :])
                    nc.scalar.activation(va[0:120, nxt:nxt + R], pz[0:120, :],
                                         sigmoid, bias=bias[:], scale=1.0)
                    # bf16 mirror for the (off-critical-path) history matmuls
                    nc.vector.tensor_copy(vbf[:, nxt:nxt + R],
                                          va[:, nxt:nxt + R])
                if b % GRP == GRP - 1:
                    # batched history for blocks 4g..4g+3 (v slots contiguous)
                    g = b // GRP
                    if g % GCHUNK == 0:
                        stage = stage_p.tile([120, GCHUNK * GW], f32,
                                             tag="stage", name="stage")
                    s0 = ((b - (GRP - 1)) % 8) * R   # 0 or 100
                    c0 = (g % GCHUNK) * GW
                    ph1 = ps_h.tile([120, GRP * R], f32, tag="ph", name="ph1")
                    nc.tensor.matmul(ph1[:], q1[:], vbf[:, s0:s0 + GRP * R],
                                     start=True, stop=True)
                    nc.vector.tensor_copy(stage[:, c0:c0 + GRP * R], ph1[:])
                    ph2 = ps_h.tile([120, GRP * R], f32, tag="ph", name="ph2")
                    nc.tensor.matmul(ph2[:], q2[:], vbf[:, s0:s0 + GRP * R],
                                     start=True, stop=True)
                    nc.vector.tensor_copy(
                        stage[:, c0 + GRP * R:c0 + 2 * GRP * R], ph2[:])
                    if g % GCHUNK == GCHUNK - 1:
                        ch = g // GCHUNK
                        nc.sync.dma_start(
                            hist_d.ap()[:, ch * GCHUNK * GW:(ch + 1) * GCHUNK * GW],
                            stage[:])

    nc.compile()
    _CACHE[key] = nc
    return nc


def kernel(init_state, step_size, A, a, B, b, c1, c2, c3, c4,
           vmax, v0, r, std_in, num_steps):
    global LAST_RESULTS
    init_state = np.asarray(init_state, np.float32)
    pars = [float(np.asarray(x)) for x in
            (step_size, A, a, B, b, c1, c2, c3, c4, vmax, v0, r, std_in)]
    (dt, A_, a_, B_, b_, c1_, c2_, c3_, c4_, vmax_, v0_, r_, std_in_) = pars
    T = int(np.asarray(num_steps))
    Rf = init_state.shape[0]

    k = K_BLK
    if Rf != R_FULL or T <= 2 * k:
        # tiny/odd shapes: host fallback (exact sequential)
        hist, _ = _seq_euler(init_state, T + 1, *pars)
        state_hist = hist[1:T + 1]
        return state_hist[-1].copy(), state_hist

    # pad T up to a multiple of GRP*GCHUNK*K_BLK (=4000)
    blk_quant = 100 * k
    T_pad = -(-T // blk_quant) * blk_quant
    nb = T_pad // k

    Z, P, Q = _build_mats(dt, A_, a_, B_, b_, c1_, c2_, c3_, c4_,
                          vmax_, v0_, r_, std_in_)
    import ml_dtypes
    Zt = np.ascontiguousarray(Z.T, np.float32)
    Q1t = np.ascontiguousarray(Q[:120].T).astype(ml_dtypes.bfloat16)
    Q2t = np.ascontiguousarray(Q[120:].T).astype(ml_dtypes.bfloat16)
    bias_np = np.full((120, 1), -r_ * v0_, np.float32)

    # bootstrap block 0 on host (exact fp32 sequential, k steps)
    states0, _ = _seq_euler(init_state, k, *pars)  # states at steps 0..k-1
    u0 = np.empty((3 * k, Rf), np.float32)
    for j in range(k):
        M = states0[j, :, 0]; E = states0[j, :, 1]; I = states0[j, :, 2]
        u0[3 * j + 0] = 1.0 / (1.0 + np.exp(-(r_ * (E - I) - r_ * v0_)))
        u0[3 * j + 1] = 1.0 / (1.0 + np.exp(-(c1_ * r_ * M - r_ * v0_)))
        u0[3 * j + 2] = 1.0 / (1.0 + np.exp(-(c3_ * r_ * M - r_ * v0_)))
    v0_full = np.concatenate(
        [u0, init_state.T.astype(np.float32), np.ones((1, Rf), np.float32)], 0)

    reps = int(os.environ.get("KERNEL_REPS", "1"))
    nc = _get_program(nb, reps)
    in_maps = []
    for c in range(N_CORES):
        sl = slice(c * R_CORE, (c + 1) * R_CORE)
        in_maps.append({
            "v0": np.ascontiguousarray(v0_full[:, sl]),
            "zw": Zt, "q1": Q1t, "q2": Q2t, "bias": bias_np,
        })

    from concourse import bass_utils
    res = bass_utils.run_bass_kernel_spmd(
        nc, in_maps, core_ids=list(range(N_CORES)),
        trace=bool(int(os.environ.get("KERNEL_TRACE", "0"))))
    LAST_RESULTS = res

    ngrp = nb // 4
    parts = []
    for c in range(N_CORES):
        # hist cols: [g, half, i, r]; rows: [j, comp]
        arr = res.results[c]["hist"].reshape(20, 6, ngrp, 2, 4, R_CORE)
        # -> [g, i, half, j, r, comp]: t = (4g+i)*40 + half*20 + j
        parts.append(arr.transpose(2, 4, 3, 0, 5, 1).reshape(
            ngrp * 4 * 40, R_CORE, 6))
    state_hist = np.concatenate(parts, axis=1)[:T]
    return state_hist[-1].copy(), state_hist
